# revision 1
# baseline (speedup 1.0000x reference)
"""Trainium2 Bass kernel for nn_BagModel_3d (segment_reduce).

Computation (per bag b):
  out[b] = (1/n_b) * sum_{i < n_b} relu(x[b, i, :] @ W1 + b1) @ W2 + b2

Strategy: data-parallel over bags, 32 bags per NeuronCore across 8 cores.
Host-side prep is layout only: shard x on the bag axis, transpose each shard
to [D_IN, bags*N_MAX] so the contraction dim lands on SBUF partitions, and
zero the padding instances (i >= n_b).

Per core, per (bag, dh-chunk): a [128, 512] PSUM tile accumulates the two
K=128 halves of z = x @ W1 (bf16 datapath, fp32 PSUM). The PSUM drain —
relu(z + b1) with a fused free-axis accumulation (the segment sum) — runs in
ONE instruction, alternating between ScalarE (activation+accum) and VectorE
(scalar_tensor_tensor+accum) so neither engine paces the loop. Zeroed
padding contributes relu(b1) per padded instance; a rank-1 (n_b-512) x
relu(b1) correction (exactly 0 for the spec's b1=0) restores the true sum.
The mean's 1/n and +b2 fold into one per-partition op on the final [32,1]
PSUM output of the W2 matmul.
"""
import sys
import numpy as np

sys.path.insert(0, '/opt/trn_rl_repo')

B, N_MAX, D_IN, D_H = 256, 512, 256, 256
N_CORES = 8
BAGS = B // N_CORES          # 32 bags per core
R = BAGS * N_MAX             # 16384 instance rows per core
GROUPS = 8                   # bag groups per core (4 bags each)
GB = BAGS // GROUPS          # bags per group = 4
GW = GB * N_MAX              # row width per group = 2048

_PROGRAM = None
_PROGRAM_KEY = None


def _build_program(b2_value):
    import concourse.bacc as bacc
    import concourse.tile as tile
    from concourse import mybir

    f32 = mybir.dt.float32
    bf16 = mybir.dt.bfloat16
    i32 = mybir.dt.int32
    Alu = mybir.AluOpType

    nc = bacc.Bacc("TRN2", target_bir_lowering=False, debug=False)

    xt = nc.dram_tensor("xt", [D_IN, R], f32, kind="ExternalInput").ap()
    n_col = nc.dram_tensor("n_col", [BAGS, 1], i32, kind="ExternalInput").ap()
    n_row = nc.dram_tensor("n_row", [1, BAGS], i32, kind="ExternalInput").ap()
    w1 = nc.dram_tensor("w1", [D_IN, D_H], f32, kind="ExternalInput").ap()
    b1 = nc.dram_tensor("b1", [D_H, 1], f32, kind="ExternalInput").ap()
    w2 = nc.dram_tensor("w2", [D_H, 1], f32, kind="ExternalInput").ap()
    out = nc.dram_tensor("out", [BAGS, 1], f32, kind="ExternalOutput").ap()

    with tile.TileContext(nc) as tc:
        with (
            tc.tile_pool(name="const", bufs=1) as cpool,
            tc.tile_pool(name="x", bufs=8) as xpool,
            tc.tile_pool(name="h", bufs=4) as hpool,
            tc.tile_pool(name="z", bufs=5, space="PSUM") as zpool,
            tc.tile_pool(name="smallps", bufs=1, space="PSUM") as spspool,
            tc.tile_pool(name="xf", bufs=2) as xf_pool,
        ):
            # ---- weights first (first matmul needs them), then x prefetch ----
            w1k0 = cpool.tile([128, D_H], bf16, tag="w1k0")
            w1k1 = cpool.tile([128, D_H], bf16, tag="w1k1")
            nc.gpsimd.dma_start(w1k0[:], w1[0:128, :])   # SWDGE f32->bf16 cast
            nc.gpsimd.dma_start(w1k1[:], w1[128:256, :])
            # Split the 16 x loads across both DGE paths: SWDGE casts f32->bf16
            # inline but its descriptor rings contend on SDMA engines 7/15;
            # HWDGE loads f32 (no ring pressure) and ACT/DVE cast on-chip.
            xtiles = []
            for g in range(GROUPS):
                pair = []
                for k in range(2):
                    xbf = xpool.tile([128, GW], bf16, tag=f"x{k}")
                    src = xt[128 * k:128 * (k + 1), GW * g:GW * (g + 1)]
                    if g % 2 == 0:
                        nc.gpsimd.dma_start(xbf[:], src)
                    else:
                        xf = xf_pool.tile([128, GW], f32, tag=f"xf{k}")
                        nc.sync.dma_start(xf[:], src)
                        if k == 0:
                            nc.scalar.copy(xbf[:], xf[:])
                        else:
                            nc.vector.tensor_copy(xbf[:], xf[:])
                    pair.append(xbf)
                xtiles.append(tuple(pair))
            b1t = cpool.tile([128, 2], f32, tag="b1t")
            nc.sync.dma_start(b1t[:, 0:1], b1[0:128, :])
            nc.sync.dma_start(b1t[:, 1:2], b1[128:256, :])
            w2t = cpool.tile([128, 2], f32, tag="w2t")
            nc.sync.dma_start(w2t[:, 0:1], w2[0:128, :])
            nc.sync.dma_start(w2t[:, 1:2], w2[128:256, :])
            zeros_t = cpool.tile([128, N_MAX], f32, tag="zeros_t")
            nc.vector.memset(zeros_t[:], 0.0)

            # ---- n-derived scalars ----
            nI_col = cpool.tile([BAGS, 1], i32, tag="nI_col")
            nc.sync.dma_start(nI_col[:], n_col[:])
            nf_col = cpool.tile([BAGS, 1], f32, tag="nf_col")
            nc.vector.tensor_copy(nf_col[:], nI_col[:])
            inv_col = cpool.tile([BAGS, 1], f32, tag="inv_col")
            nc.vector.reciprocal(inv_col[:], nf_col[:])

            # padding correction: corr_c = relu(b1_c) (x) (n - 512)  [128, BAGS]
            nI_row = cpool.tile([1, BAGS], i32, tag="nI_row")
            nc.sync.dma_start(nI_row[:], n_row[:])
            nf_row = cpool.tile([1, BAGS], f32, tag="nf_row")
            nc.vector.tensor_copy(nf_row[:], nI_row[:])
            cnt_row = cpool.tile([1, BAGS], f32, tag="cnt_row")
            nc.vector.tensor_scalar(cnt_row[:], nf_row[:], 512.0, None,
                                    op0=Alu.subtract)
            b1row = cpool.tile([1, D_H], f32, tag="b1row")
            nc.sync.dma_start(b1row[:], b1[:, :])
            rb1row = cpool.tile([1, D_H], f32, tag="rb1row")
            nc.vector.tensor_scalar(rb1row[:], b1row[:], 0.0, None, op0=Alu.max)

            praw0 = cpool.tile([128, BAGS], f32, tag="praw0")
            praw1 = cpool.tile([128, BAGS], f32, tag="praw1")
            praws = (praw0, praw1)

            # ---- main loop ----
            for g in range(GROUPS):
                x0, x1 = xtiles[g]
                for j in range(GB):
                    b = GB * g + j
                    for c in range(2):
                        z = zpool.tile([128, N_MAX], f32, tag="z")
                        nc.tensor.matmul(
                            z[:], w1k0[:, 128 * c:128 * (c + 1)],
                            x0[:, N_MAX * j:N_MAX * (j + 1)],
                            start=True, stop=False)
                        nc.tensor.matmul(
                            z[:], w1k1[:, 128 * c:128 * (c + 1)],
                            x1[:, N_MAX * j:N_MAX * (j + 1)],
                            start=False, stop=True)
                        h = hpool.tile([128, N_MAX], f32, tag="h")
                        if c == 0:
                            # ScalarE: relu(z + b1) with fused row-sum
                            nc.scalar.activation(
                                h[:], z[:], mybir.ActivationFunctionType.Relu,
                                bias=b1t[:, c:c + 1], scale=1.0,
                                accum_out=praws[c][:, b:b + 1])
                        else:
                            # VectorE: max(z + b1, 0) with fused row-sum
                            nc.vector.scalar_tensor_tensor(
                                h[:], z[:], b1t[:, c:c + 1], zeros_t[:],
                                op0=Alu.add, op1=Alu.max,
                                accum_out=praws[c][:, b:b + 1])

            # ---- padding correction + final Linear ----
            pscs = []
            for c in range(2):
                corr = spspool.tile([128, BAGS], f32, tag=f"corr{c}")
                nc.tensor.matmul(corr[:], rb1row[0:1, 128 * c:128 * (c + 1)],
                                 cnt_row[:], start=True, stop=True)
                psc = cpool.tile([128, BAGS], f32, tag=f"psc{c}")
                nc.vector.tensor_add(psc[:], praws[c][:], corr[:])
                pscs.append(psc)
            po = spspool.tile([BAGS, 1], f32, tag="po")
            nc.tensor.matmul(po[:], pscs[0][:], w2t[:, 0:1], start=True, stop=False)
            nc.tensor.matmul(po[:], pscs[1][:], w2t[:, 1:2], start=False, stop=True)
            osb = cpool.tile([BAGS, 1], f32, tag="osb")
            nc.vector.tensor_scalar(
                osb[:], po[:], inv_col[:, 0:1], float(b2_value),
                op0=Alu.mult, op1=Alu.add)
            nc.sync.dma_start(out[:], osb[:])

    nc.compile()
    return nc


def get_program(b2_value=0.0):
    global _PROGRAM, _PROGRAM_KEY
    key = float(b2_value)
    if _PROGRAM is None or _PROGRAM_KEY != key:
        _PROGRAM = _build_program(key)
        _PROGRAM_KEY = key
    return _PROGRAM


def make_in_maps(x, n_instances, W1, b1, W2, b2=None):
    x = np.asarray(x, dtype=np.float32)
    n = np.asarray(n_instances, dtype=np.int32)
    W1 = np.asarray(W1, dtype=np.float32)
    b1 = np.asarray(b1, dtype=np.float32).reshape(D_H, 1)
    W2 = np.asarray(W2, dtype=np.float32).reshape(D_H, 1)
    in_maps = []
    for c in range(N_CORES):
        xs = x[c * BAGS:(c + 1) * BAGS]              # [32, 512, 256]
        xt = np.ascontiguousarray(xs.transpose(2, 0, 1).reshape(D_IN, R))
        ns = n[c * BAGS:(c + 1) * BAGS]
        for i in range(BAGS):                        # zero padding instances
            xt[:, i * N_MAX + int(ns[i]):(i + 1) * N_MAX] = 0.0
        in_maps.append({
            "xt": xt,
            "n_col": np.ascontiguousarray(ns.reshape(BAGS, 1)),
            "n_row": np.ascontiguousarray(ns.reshape(1, BAGS)),
            "w1": W1, "b1": b1, "w2": W2,
        })
    return in_maps


def run_spmd(in_maps, b2_value=0.0, trace=False, **kwargs):
    from concourse import bass_utils
    if trace:
        # no S3 in this environment; keep trace artifacts local
        bass_utils.upload_artifacts = lambda tmpdir: tmpdir
    nc = get_program(b2_value)
    return bass_utils.run_bass_kernel_spmd(
        nc, in_maps, core_ids=list(range(N_CORES)), trace=trace, **kwargs)


def kernel(x, n_instances, W1, b1, W2, b2):
    b2_value = float(np.asarray(b2).reshape(-1)[0])
    in_maps = make_in_maps(x, n_instances, W1, b1, W2, b2)
    res = run_spmd(in_maps, b2_value=b2_value)
    return np.concatenate([res.results[c]["out"] for c in range(N_CORES)], axis=0)



# revision 5
# speedup vs baseline: 1.4665x; 1.4665x over previous
"""Trainium2 Bass kernel for nn_BagModel_3d (segment_reduce).

Computation (per bag b):
  out[b] = (1/n_b) * sum_{i < n_b} relu(x[b, i, :] @ W1 + b1) @ W2 + b2

Strategy: data-parallel over bags, 32 bags per NeuronCore across 8 cores.
Host-side prep is layout only: shard x on the bag axis and rearrange each
shard to [128, bag, k, inst] so each bag's data is one contiguous 512 KB
block with the contraction dim split into two 128-row halves on SBUF
partitions; padding instances (i >= n_b) are zeroed.

The kernel streams x bag-by-bag on the sync HWDGE ring (512 KB per DMA,
~6 bags prefetched), runs the K=256 contraction as two float32r matmuls
per 128-wide dh chunk (fp32r streams 1 row/cycle for free dim >= 256 - no
bf16 cast needed anywhere), and drains each PSUM bank once with a fused
relu(z + b1) + free-axis accumulation: dh chunk 0 on ScalarE (activation),
chunk 1 on VectorE (tensor_scalar), so the two drain engines split the
work. Zeroed padding contributes relu(b1) per padded instance; a rank-1
(n_b-512) x relu(b1) correction (exactly 0 for the spec's b1=0) restores
the true sum. The mean's 1/n and +b2 fold into one per-partition op on the
final [32,1] PSUM output of the W2 matmul.
"""
import sys
import numpy as np

sys.path.insert(0, '/opt/trn_rl_repo')

B, N_MAX, D_IN, D_H = 256, 512, 256, 256
N_CORES = 8
BAGS = B // N_CORES          # 32 bags per core
PF = 6                       # x-DMA prefetch depth (bags)
XBUFS = 8                    # x tile ring depth

_PROGRAM = None
_PROGRAM_KEY = None


def _build_program(b2_value):
    import concourse.bacc as bacc
    import concourse.tile as tile
    from concourse import mybir

    f32 = mybir.dt.float32
    f32r = mybir.dt.float32r
    i32 = mybir.dt.int32
    Alu = mybir.AluOpType
    Act = mybir.ActivationFunctionType

    nc = bacc.Bacc("TRN2", target_bir_lowering=False, debug=False)

    xk = nc.dram_tensor("xk", [128, BAGS * 1024], f32r, kind="ExternalInput").ap()
    n_col = nc.dram_tensor("n_col", [BAGS, 1], i32, kind="ExternalInput").ap()
    n_row = nc.dram_tensor("n_row", [1, BAGS], i32, kind="ExternalInput").ap()
    w1 = nc.dram_tensor("w1", [D_IN, D_H], f32r, kind="ExternalInput").ap()
    b1 = nc.dram_tensor("b1", [D_H, 1], f32, kind="ExternalInput").ap()
    w2 = nc.dram_tensor("w2", [D_H, 1], f32, kind="ExternalInput").ap()
    out = nc.dram_tensor("out", [BAGS, 1], f32, kind="ExternalOutput").ap()

    with tile.TileContext(nc) as tc:
        with (
            tc.tile_pool(name="const", bufs=1) as cpool,
            tc.tile_pool(name="x", bufs=XBUFS) as xpool,
            tc.tile_pool(name="h", bufs=3) as hpool,
            tc.tile_pool(name="z", bufs=6, space="PSUM") as zpool,
            tc.tile_pool(name="smallps", bufs=1, space="PSUM") as spspool,
        ):
            # ---- weights first (first matmul needs them) ----
            w1k0 = cpool.tile([128, D_H], f32r, tag="w1k0")
            w1k1 = cpool.tile([128, D_H], f32r, tag="w1k1")
            nc.sync.dma_start(w1k0[:], w1[0:128, :])
            nc.scalar.dma_start(w1k1[:], w1[128:256, :])

            # small tensors on gpsimd SWDGE (keeps the HWDGE rings for x)
            b1t = cpool.tile([128, 2], f32, tag="b1t")
            nc.gpsimd.dma_start(b1t[:, 0:1], b1[0:128, :])
            nc.gpsimd.dma_start(b1t[:, 1:2], b1[128:256, :])
            w2t = cpool.tile([128, 2], f32, tag="w2t")
            nc.gpsimd.dma_start(w2t[:, 0:1], w2[0:128, :])
            nc.gpsimd.dma_start(w2t[:, 1:2], w2[128:256, :])
            nI_col = cpool.tile([BAGS, 1], i32, tag="nI_col")
            nc.gpsimd.dma_start(nI_col[:], n_col[:])
            nI_row = cpool.tile([1, BAGS], i32, tag="nI_row")
            nc.gpsimd.dma_start(nI_row[:], n_row[:])
            b1row = cpool.tile([1, D_H], f32, tag="b1row")
            nc.gpsimd.dma_start(b1row[:], b1.transpose([1, 0]))

            # ---- n-derived scalars (vector, during DMA fill) ----
            nf_col = cpool.tile([BAGS, 1], f32, tag="nf_col")
            nc.vector.tensor_copy(nf_col[:], nI_col[:])
            inv_col = cpool.tile([BAGS, 1], f32, tag="inv_col")
            nc.vector.reciprocal(inv_col[:], nf_col[:])
            nf_row = cpool.tile([1, BAGS], f32, tag="nf_row")
            nc.vector.tensor_copy(nf_row[:], nI_row[:])
            cnt_row = cpool.tile([1, BAGS], f32, tag="cnt_row")
            nc.vector.tensor_scalar(cnt_row[:], nf_row[:], 512.0, None,
                                    op0=Alu.subtract)
            rb1row = cpool.tile([1, D_H], f32, tag="rb1row")
            nc.vector.tensor_scalar(rb1row[:], b1row[:], 0.0, None, op0=Alu.max)

            praw0 = cpool.tile([128, BAGS], f32, tag="praw0")
            praw1 = cpool.tile([128, BAGS], f32, tag="praw1")
            zeros_t = cpool.tile([128, N_MAX], f32, tag="zeros_t")
            nc.vector.memset(zeros_t[:], 0.0)

            w10r = w1k0
            w11r = w1k1

            # ---- x stream + per-bag pipeline ----
            xtiles = [None] * BAGS

            def issue_x(b):
                xb = xpool.tile([128, 1024], f32r, tag="x", name=f"x_{b}")
                nc.sync.dma_start(xb[:], xk[:, 1024 * b:1024 * (b + 1)])
                xtiles[b] = xb

            for b in range(min(PF, BAGS)):
                issue_x(b)

            for b in range(BAGS):
                if b + PF < BAGS:
                    issue_x(b + PF)
                xb = xtiles[b]
                x0 = xb[:, 0:512]
                x1 = xb[:, 512:1024]
                z0 = zpool.tile([128, N_MAX], f32, tag="z", name=f"z0_{b}")
                nc.tensor.matmul(z0[:], w10r[:, 0:128], x0, start=True, stop=False)
                nc.tensor.matmul(z0[:], w11r[:, 0:128], x1, start=False, stop=True)
                z1 = zpool.tile([128, N_MAX], f32, tag="z", name=f"z1_{b}")
                nc.tensor.matmul(z1[:], w10r[:, 128:256], x0, start=True, stop=False)
                nc.tensor.matmul(z1[:], w11r[:, 128:256], x1, start=False, stop=True)
                # ScalarE: relu(z + b1) with fused row-sum (dh chunk 0)
                h0 = hpool.tile([128, N_MAX], f32, tag="hs", name=f"h0_{b}")
                nc.scalar.activation(h0[:], z0[:], Act.Relu,
                                     bias=b1t[:, 0:1], scale=1.0,
                                     accum_out=praw0[:, b:b + 1])
                # VectorE: max(z + b1, 0) with fused row-sum (dh chunk 1)
                h1 = hpool.tile([128, N_MAX], f32, tag="hv", name=f"h1_{b}")
                nc.vector.scalar_tensor_tensor(
                    h1[:], z1[:], b1t[:, 1:2], zeros_t[:],
                    op0=Alu.add, op1=Alu.max,
                    accum_out=praw1[:, b:b + 1])

            # ---- padding correction + final Linear ----
            pscs = []
            for c in range(2):
                corr = spspool.tile([128, BAGS], f32, tag="corr", name=f"corr{c}")
                nc.tensor.matmul(corr[:], rb1row[0:1, 128 * c:128 * (c + 1)],
                                 cnt_row[:], start=True, stop=True)
                psc = cpool.tile([128, BAGS], f32, tag=f"psc{c}")
                nc.vector.tensor_add(psc[:], (praw0, praw1)[c][:], corr[:])
                pscs.append(psc)
            po = spspool.tile([BAGS, 1], f32, tag="po")
            nc.tensor.matmul(po[:], pscs[0][:], w2t[:, 0:1], start=True, stop=False)
            nc.tensor.matmul(po[:], pscs[1][:], w2t[:, 1:2], start=False, stop=True)
            osb = cpool.tile([BAGS, 1], f32, tag="osb")
            nc.vector.tensor_scalar(
                osb[:], po[:], inv_col[:, 0:1], float(b2_value),
                op0=Alu.mult, op1=Alu.add)
            nc.sync.dma_start(out[:], osb[:])

    nc.compile()
    return nc


def get_program(b2_value=0.0):
    global _PROGRAM, _PROGRAM_KEY
    key = float(b2_value)
    if _PROGRAM is None or _PROGRAM_KEY != key:
        _PROGRAM = _build_program(key)
        _PROGRAM_KEY = key
    return _PROGRAM


def make_in_maps(x, n_instances, W1, b1, W2, b2=None):
    x = np.asarray(x, dtype=np.float32)
    n = np.asarray(n_instances, dtype=np.int32)
    W1 = np.asarray(W1, dtype=np.float32)
    b1 = np.asarray(b1, dtype=np.float32).reshape(D_H, 1)
    W2 = np.asarray(W2, dtype=np.float32).reshape(D_H, 1)
    in_maps = []
    for c in range(N_CORES):
        xs = x[c * BAGS:(c + 1) * BAGS]              # [32, 512, 256]
        # [128(p), bag, k, inst]: din = 128k + p, bag-contiguous blocks
        arr = xs.transpose(2, 0, 1).reshape(2, 128, BAGS, N_MAX)
        arr = np.ascontiguousarray(arr.transpose(1, 2, 0, 3))
        ns = n[c * BAGS:(c + 1) * BAGS]
        for i in range(BAGS):                        # zero padding instances
            arr[:, i, :, int(ns[i]):] = 0.0
        in_maps.append({
            "xk": arr.reshape(128, BAGS * 1024),
            "n_col": np.ascontiguousarray(ns.reshape(BAGS, 1)),
            "n_row": np.ascontiguousarray(ns.reshape(1, BAGS)),
            "w1": W1, "b1": b1, "w2": W2,
        })
    return in_maps


def run_spmd(in_maps, b2_value=0.0, trace=False, **kwargs):
    from concourse import bass_utils
    if trace:
        # no S3 in this environment; keep trace artifacts local
        bass_utils.upload_artifacts = lambda tmpdir: tmpdir
    nc = get_program(b2_value)
    return bass_utils.run_bass_kernel_spmd(
        nc, in_maps, core_ids=list(range(N_CORES)), trace=trace, **kwargs)


def kernel(x, n_instances, W1, b1, W2, b2):
    b2_value = float(np.asarray(b2).reshape(-1)[0])
    in_maps = make_in_maps(x, n_instances, W1, b1, W2, b2)
    res = run_spmd(in_maps, b2_value=b2_value)
    return np.concatenate([res.results[c]["out"] for c in range(N_CORES)], axis=0)
